# revision 14
# baseline (speedup 1.0000x reference)
"""Trainium2 Bass kernel for a ragged-length LSTM encoder.

Problem: B=64, T=2048, E=H=256 LSTM with per-row lengths; outputs [B,T,H]
(zeros past each row's length) plus final (h, c) state frozen at length-1.

Strategy (data-parallel, 8 rows per NeuronCore):
  - Everything on-chip in "transposed" layout: hidden dim on partitions,
    batch on the free dim, so elementwise gate math runs on [128, 16] tiles.
  - Recurrent matmul z_h^T = W_h^T @ h^T as 16 (2 K-tiles x 8 M-tiles)
    [128,128]x[128,8] matmuls per step with fp16 weights (fast weight load).
  - Input projection U = x @ W_x + b_eff precomputed on-device in 32-step
    chunks (fp32 matmuls off the critical path, double buffered).
  - tanh(j) = 2*sigmoid(2j) - 1 with j columns of W/b pre-scaled by 2, so a
    single Sigmoid activation covers all four gates; one Tanh for c.
  - No masking/freezing on device: rows past their length keep evolving
    harmlessly (batch rows are independent). Host zeroes output tails and
    gathers the final h from the output history and final c from a c-history
    that is DMA'd out raw.
"""

import os
import numpy as np

import concourse.bass as bass
import concourse.mybir as mybir
import concourse.tile as tile
from concourse import bacc
from concourse.bass import ds

AF = mybir.ActivationFunctionType
ALU = mybir.AluOpType
F32 = mybir.dt.float32
F16 = mybir.dt.float16

P = 128
B_LOC = 8       # batch rows per core
E = 256         # input embedding dim
H = 256         # hidden dim
G4 = 1024       # 4*H
CH = 32         # timesteps per chunk
N_CORES = 8

_CACHE = {}


def build_nc(T):
    """Build the single-core SPMD program (same program runs on all 8 cores)."""
    NCH = T // CH           # number of chunks
    ITERS = NCH // 2        # For_i iterations (2 chunks per body)
    TPAD = T + 2 * CH       # x is padded so the next-chunk prefetch never OOBs
    UCOLS = B_LOC * CH      # 256 columns per U chunk (b-major: col = b*CH + t)

    nc = bacc.Bacc(trn_type="TRN2")

    x_pad = nc.dram_tensor("x_pad", [B_LOC, TPAD, E], F32, kind="ExternalInput")
    wx = nc.dram_tensor("wx", [2, 8, P, P], F32, kind="ExternalInput")
    wh = nc.dram_tensor("wh", [2, 8, P, P], F16, kind="ExternalInput")
    bcol = nc.dram_tensor("bcol", [8, P], F32, kind="ExternalInput")
    ident = nc.dram_tensor("ident", [P, P], F32, kind="ExternalInput")

    out = nc.dram_tensor("out", [B_LOC, T, H], F32, kind="ExternalOutput")
    c_raw = nc.dram_tensor("c_raw", [NCH, P, 2 * B_LOC * CH], F32,
                           kind="ExternalOutput")

    with tile.TileContext(nc) as tc:
        with (
            tc.tile_pool(name="persist", bufs=1) as pp,
            tc.tile_pool(name="step", bufs=3) as sp,
            tc.tile_pool(name="xs", bufs=2) as xp,
            tc.tile_pool(name="zpsum", bufs=2, space="PSUM") as zp,
            tc.tile_pool(name="upsum", bufs=2, space="PSUM") as up,
            tc.tile_pool(name="tpsum", bufs=2, space="PSUM") as tp,
        ):
            # ---- persistent tiles ----
            WXs = pp.tile([P, 2, 8, P], F32, name="WXs")
            WHs = pp.tile([P, 2, 8, P], F16, name="WHs")
            BS = pp.tile([P, 8], F32, name="BS")
            IDs = pp.tile([P, P], F32, name="IDs")
            UA = pp.tile([P, 8, B_LOC, CH], F32, name="UA")
            UB = pp.tile([P, 8, B_LOC, CH], F32, name="UB")
            cTe = pp.tile([P, 2, B_LOC, CH], F32, name="cTe")
            cTo = pp.tile([P, 2, B_LOC, CH], F32, name="cTo")
            oTe = pp.tile([P, 2, B_LOC, CH], F32, name="oTe")
            oTo = pp.tile([P, 2, B_LOC, CH], F32, name="oTo")
            hX = pp.tile([P, 2, B_LOC], F16, name="hX")
            hY = pp.tile([P, 2, B_LOC], F16, name="hY")

            nc.sync.dma_start(WXs[:], wx[:].rearrange("kt mt k m -> k kt mt m"))
            nc.sync.dma_start(WHs[:], wh[:].rearrange("kt mt k m -> k kt mt m"))
            nc.sync.dma_start(BS[:], bcol[:].rearrange("mt k -> k mt"))
            nc.sync.dma_start(IDs[:], ident[:])
            nc.vector.memset(hX[:], 0.0)
            nc.vector.memset(cTo[:], 0.0)
            # Collapse the constant-load DMA waits so downstream PE
            # instructions don't exceed the HW sync-wait slot limit.
            tc.strict_bb_all_engine_barrier()

            # ---- helpers ----
            def dma_x_chunk(t0_sv):
                """DMA one chunk of x into two [128, 256] (b,t)-major tiles."""
                tiles = []
                for bg in range(2):
                    xn = xp.tile([P, E], F32, tag=f"xn{bg}")
                    src = x_pad[bg * 4:(bg + 1) * 4, :, :][:, ds(t0_sv, CH), :]
                    nc.sync.dma_start(xn[:], src)
                    tiles.append(xn)
                return tiles

            def make_xT(xn_tiles):
                """PE-transpose x chunk tiles into xT [128(e-half), 2, 256]."""
                xT = xp.tile([P, 2, UCOLS], F32, tag="xT")
                for bg in range(2):
                    for kt in range(2):
                        ps = tp.tile([P, P], F32, tag="trps")
                        nc.tensor.transpose(
                            ps[:], xn_tiles[bg][:, kt * P:(kt + 1) * P], IDs[:])
                        nc.vector.tensor_copy(
                            xT[:, kt, bg * P:(bg + 1) * P], ps[:])
                return xT

            def u_piece(xT, U, mt):
                """One M-tile of the U-chunk matmul + bias copy."""
                psu = up.tile([P, UCOLS], F32, tag="upsum")
                nc.tensor.matmul(psu[:], WXs[:, 0, mt, :], xT[:, 0, :],
                                 start=True, stop=False)
                nc.tensor.matmul(psu[:], WXs[:, 1, mt, :], xT[:, 1, :],
                                 start=False, stop=True)
                nc.vector.tensor_scalar(
                    out=U[:, mt, :, :].rearrange("p b t -> p (b t)"),
                    in0=psu[:], scalar1=BS[:, mt:mt + 1],
                    scalar2=None, op0=ALU.add)

            def step(t, U, cT, cT_prev_buf, g_step):
                """One LSTM timestep (t = index within chunk)."""
                h_prev = hX if g_step % 2 == 0 else hY
                h_new = hY if g_step % 2 == 0 else hX
                c_prev = (cT[:, :, :, t - 1] if t > 0
                          else cT_prev_buf[:, :, :, CH - 1])

                zps = zp.tile([P, 64], F32, tag="zps")
                for mt in range(8):
                    for kt in range(2):
                        nc.tensor.matmul(
                            zps[:, mt * 8:(mt + 1) * 8],
                            WHs[:, kt, mt, :],
                            h_prev[:, kt, :],
                            start=(kt == 0), stop=(kt == 1))

                zbuf = sp.tile([P, 4, 2, B_LOC], F32, tag="zbuf")
                uslice = U[:, :, :, t].rearrange("p (g h) b -> p g h b", h=2)
                nc.vector.tensor_tensor(
                    zbuf[:], zps[:].rearrange("p (g h b) -> p g h b", h=2, b=B_LOC),
                    uslice, ALU.add)

                sg = sp.tile([P, 4, 2, B_LOC], F32, tag="sg")
                nc.scalar.activation(sg[:], zbuf[:], AF.Sigmoid)
                gi, gj, gf, go = sg[:, 0], sg[:, 1], sg[:, 2], sg[:, 3]

                t1 = sp.tile([P, 2, B_LOC], F32, tag="t1")
                nc.vector.tensor_mul(t1[:], gi, gj)
                d = sp.tile([P, 2, B_LOC], F32, tag="d")
                nc.vector.scalar_tensor_tensor(
                    d[:], t1[:], 2.0, gi, ALU.mult, ALU.subtract)
                ca = sp.tile([P, 2, B_LOC], F32, tag="ca")
                nc.gpsimd.tensor_mul(ca[:], c_prev, gf)
                nc.vector.tensor_add(cT[:, :, :, t], ca[:], d[:])

                th = sp.tile([P, 2, B_LOC], F32, tag="th")
                nc.scalar.activation(th[:], cT[:, :, :, t], AF.Tanh)
                nc.vector.tensor_mul(h_new[:], th[:], go)
                return h_new

            def out_store(oT, t0_sv):
                """Transpose the chunk's h-history and DMA to `out`."""
                for bg in range(2):
                    stg = xp.tile([P, H], F32, tag=f"stg{bg}")
                    for bl in range(4):
                        b = bg * 4 + bl
                        for half in range(2):
                            ps = tp.tile([CH, P], F32, tag="otps")
                            nc.tensor.transpose(ps[:], oT[:, half, b, :], IDs[:])
                            nc.vector.tensor_copy(
                                stg[bl * CH:(bl + 1) * CH,
                                    half * P:(half + 1) * P], ps[:])
                    dst = out[bg * 4:(bg + 1) * 4, :, :][:, ds(t0_sv, CH), :]
                    nc.sync.dma_start(dst, stg[:])

            def write_oT(oT, t, h_new):
                nc.gpsimd.tensor_copy(oT[:, :, :, t], h_new[:])

            # ---- prologue: build U for chunk 0 ----
            xt0 = dma_x_chunk(0)
            xT0 = make_xT(xt0)
            for mt in range(8):
                u_piece(xT0, UA, mt)

            # ---- main loop: one iteration = chunks (2i, 2i+1) ----
            with tc.For_i(0, ITERS, 1,
                          hint_engines=(mybir.EngineType.PE,
                                        mybir.EngineType.DVE)) as it:
                t0e = it * (2 * CH)
                t0o = it * (2 * CH) + CH
                t0n = it * (2 * CH) + 2 * CH

                # even chunk; build UB (for the odd chunk) alongside
                xt_o = dma_x_chunk(t0o)
                xT_o = None
                for t in range(CH):
                    hn = step(t, UA, cTe, cTo, t)
                    write_oT(oTe, t, hn)
                    if t == 2:
                        xT_o = make_xT(xt_o)
                    if 4 <= t < 12:
                        u_piece(xT_o, UB, t - 4)

                # odd chunk; build UA for the next iteration's even chunk,
                # and store the even chunk's outputs alongside
                xt_n = dma_x_chunk(t0n)
                xT_n = None
                for t in range(CH):
                    hn = step(t, UB, cTo, cTe, t)
                    write_oT(oTo, t, hn)
                    if t == 2:
                        xT_n = make_xT(xt_n)
                    if 4 <= t < 12:
                        u_piece(xT_n, UA, t - 4)
                    if t == 14:
                        out_store(oTe, t0e)

                out_store(oTo, t0o)
                nc.sync.dma_start(
                    c_raw[ds(it * 2, 1)],
                    cTe[:].rearrange("p h b t -> p (h b t)"))
                nc.sync.dma_start(
                    c_raw[ds(it * 2 + 1, 1)],
                    cTo[:].rearrange("p h b t -> p (h b t)"))

    return nc


def _prep_weights(W, b):
    """Host-side weight prep: split/scale/reorder into device layouts."""
    W = np.asarray(W, np.float32)
    b = np.asarray(b, np.float32)
    Wx = W[:E].copy()            # [256, 1024]
    Wh = W[E:].copy()            # [256, 1024]
    b_eff = b.copy()
    # j gate (cols 256:512) pre-scaled by 2 for tanh(j) = 2*sigmoid(2j) - 1
    Wx[:, 256:512] *= 2.0
    Wh[:, 256:512] *= 2.0
    b_eff[256:512] *= 2.0
    # f gate (cols 512:768): reference uses sigmoid(f + 1)
    b_eff[512:768] += 1.0

    # Build [2, 8, 128, 128] tile arrays: arr[kt, mt] = M[kt*128:(kt+1)*128, mt*128:(mt+1)*128]
    def to_tiles(M, dt):
        a = M.reshape(2, P, 8, P).transpose(0, 2, 1, 3)
        return np.ascontiguousarray(a).astype(dt)

    wx_t = to_tiles(Wx, np.float32)
    wh_t = to_tiles(Wh, np.float16)
    bcol = np.ascontiguousarray(b_eff.reshape(8, P)).astype(np.float32)
    return wx_t, wh_t, bcol


def kernel(input_embeddings, input_length, W, b):
    from concourse.bass_utils import run_bass_kernel_spmd

    x = np.asarray(input_embeddings, np.float32)
    lengths = np.asarray(input_length, np.int32)
    B, T, _ = x.shape
    assert B == N_CORES * B_LOC

    key = T
    if key not in _CACHE:
        nc_new = build_nc(T)
        if not nc_new.is_finalized():
            nc_new.finalize()
        _CACHE[key] = nc_new
    nc = _CACHE[key]

    wx_t, wh_t, bcol = _prep_weights(W, b)
    ident = np.eye(P, dtype=np.float32)
    TPAD = T + 2 * CH

    in_maps = []
    for k in range(N_CORES):
        xs = np.zeros((B_LOC, TPAD, E), np.float32)
        xs[:, :T] = x[k * B_LOC:(k + 1) * B_LOC]
        in_maps.append({
            "x_pad": xs, "wx": wx_t, "wh": wh_t, "bcol": bcol, "ident": ident,
        })

    trace = os.environ.get("KERNEL_PROFILE", "0") == "1"
    try:
        res = run_bass_kernel_spmd(nc, in_maps, list(range(N_CORES)), trace=trace)
    except ModuleNotFoundError:
        res = run_bass_kernel_spmd(nc, in_maps, list(range(N_CORES)), trace=False)
    global LAST_EXEC_NS
    LAST_EXEC_NS = res.exec_time_ns

    NCH = T // CH
    outputs = np.concatenate([res.results[k]["out"] for k in range(N_CORES)], axis=0)
    h_fin = np.zeros((B, H), np.float32)
    c_fin = np.zeros((B, H), np.float32)
    for bi in range(B):
        L = int(lengths[bi])
        if L > 0:
            h_fin[bi] = outputs[bi, L - 1]
            core, bl = bi // B_LOC, bi % B_LOC
            craw = res.results[core]["c_raw"]  # [NCH, 128, 512]
            ci, tr = (L - 1) // CH, (L - 1) % CH
            for half in range(2):
                c_fin[bi, half * P:(half + 1) * P] = \
                    craw[ci, :, half * (B_LOC * CH) + bl * CH + tr]
        outputs[bi, L:] = 0.0

    return outputs, (h_fin, c_fin)


LAST_EXEC_NS = None


# revision 15
# speedup vs baseline: 20494.5754x; 20494.5754x over previous
"""Trainium2 Bass kernel for a ragged-length LSTM encoder.

Problem: B=64, T=2048, E=H=256 LSTM with per-row lengths; outputs [B,T,H]
(zeros past each row's length) plus final (h, c) state frozen at length-1.

Strategy (data-parallel, 8 rows per NeuronCore):
  - Everything on-chip in "transposed" layout: hidden dim on partitions,
    batch on the free dim, so elementwise gate math runs on [128, 16] tiles.
  - Recurrent matmul z_h^T = W_h^T @ h^T as 16 (2 K-tiles x 8 M-tiles)
    [128,128]x[128,8] matmuls per step with fp16 weights (fast weight load).
  - Input projection U = x @ W_x + b_eff precomputed on-device in 32-step
    chunks (fp32 matmuls off the critical path, double buffered).
  - tanh(j) = 2*sigmoid(2j) - 1 with j columns of W/b pre-scaled by 2, so a
    single Sigmoid activation covers all four gates; one Tanh for c.
  - No masking/freezing on device: rows past their length keep evolving
    harmlessly (batch rows are independent). Host zeroes output tails and
    gathers the final h from the output history and final c from a c-history
    that is DMA'd out raw.
"""

import os
import numpy as np

import concourse.bass as bass
import concourse.mybir as mybir
import concourse.tile as tile
from concourse import bacc
from concourse.bass import ds

AF = mybir.ActivationFunctionType
ALU = mybir.AluOpType
F32 = mybir.dt.float32
F16 = mybir.dt.float16

P = 128
B_LOC = 8       # batch rows per core
E = 256         # input embedding dim
H = 256         # hidden dim
G4 = 1024       # 4*H
CH = 32         # timesteps per chunk
N_CORES = 8

_CACHE = {}


def build_nc(T):
    """Build the single-core SPMD program (same program runs on all 8 cores)."""
    NCH = T // CH           # number of chunks
    ITERS = NCH // 2        # For_i iterations (2 chunks per body)
    TPAD = T + 2 * CH       # x is padded so the next-chunk prefetch never OOBs
    UCOLS = B_LOC * CH      # 256 columns per U chunk (b-major: col = b*CH + t)

    nc = bacc.Bacc(trn_type="TRN2")

    x_pad = nc.dram_tensor("x_pad", [B_LOC, TPAD, E], F32, kind="ExternalInput")
    wx = nc.dram_tensor("wx", [2, 8, P, P], F32, kind="ExternalInput")
    wh = nc.dram_tensor("wh", [2, 8, P, P], F16, kind="ExternalInput")
    bcol = nc.dram_tensor("bcol", [8, P], F32, kind="ExternalInput")
    ident = nc.dram_tensor("ident", [P, P], F32, kind="ExternalInput")

    out = nc.dram_tensor("out", [B_LOC, T, H], F32, kind="ExternalOutput")
    c_raw = nc.dram_tensor("c_raw", [NCH, P, 2 * B_LOC * CH], F32,
                           kind="ExternalOutput")

    with tile.TileContext(nc) as tc:
        with (
            tc.tile_pool(name="persist", bufs=1) as pp,
            tc.tile_pool(name="step", bufs=3) as sp,
            tc.tile_pool(name="xs", bufs=2) as xp,
            tc.tile_pool(name="zpsum", bufs=2, space="PSUM") as zp,
            tc.tile_pool(name="upsum", bufs=2, space="PSUM") as up,
            tc.tile_pool(name="tpsum", bufs=2, space="PSUM") as tp,
        ):
            # ---- persistent tiles ----
            WXs = pp.tile([P, 2, 8, P], F32, name="WXs")
            WHs = pp.tile([P, 2, 8, P], F16, name="WHs")
            BS = pp.tile([P, 8], F32, name="BS")
            IDs = pp.tile([P, P], F32, name="IDs")
            UA = pp.tile([P, 8, B_LOC, CH], F32, name="UA")
            UB = pp.tile([P, 8, B_LOC, CH], F32, name="UB")
            cTe = pp.tile([P, 2, B_LOC, CH], F32, name="cTe")
            cTo = pp.tile([P, 2, B_LOC, CH], F32, name="cTo")
            oTe = pp.tile([P, 2, B_LOC, CH], F32, name="oTe")
            oTo = pp.tile([P, 2, B_LOC, CH], F32, name="oTo")
            hX = pp.tile([P, 2, B_LOC], F16, name="hX")
            hY = pp.tile([P, 2, B_LOC], F16, name="hY")

            nc.sync.dma_start(WXs[:], wx[:].rearrange("kt mt k m -> k kt mt m"))
            nc.sync.dma_start(WHs[:], wh[:].rearrange("kt mt k m -> k kt mt m"))
            nc.sync.dma_start(BS[:], bcol[:].rearrange("mt k -> k mt"))
            nc.sync.dma_start(IDs[:], ident[:])
            nc.vector.memset(hX[:], 0.0)
            nc.vector.memset(cTo[:], 0.0)
            # Collapse the constant-load DMA waits so downstream PE
            # instructions don't exceed the HW sync-wait slot limit.
            tc.strict_bb_all_engine_barrier()

            # ---- helpers ----
            def dma_x_chunk(t0_sv):
                """DMA one chunk of x into two [128, 256] (b,t)-major tiles."""
                tiles = []
                for bg in range(2):
                    xn = xp.tile([P, E], F32, tag=f"xn{bg}")
                    src = x_pad[bg * 4:(bg + 1) * 4, :, :][:, ds(t0_sv, CH), :]
                    nc.sync.dma_start(xn[:], src)
                    tiles.append(xn)
                return tiles

            def make_xT(xn_tiles):
                """PE-transpose x chunk tiles into xT [128(e-half), 2, 256]."""
                xT = xp.tile([P, 2, UCOLS], F32, tag="xT")
                for bg in range(2):
                    for kt in range(2):
                        ps = tp.tile([P, P], F32, tag="trps")
                        nc.tensor.transpose(
                            ps[:], xn_tiles[bg][:, kt * P:(kt + 1) * P], IDs[:])
                        nc.vector.tensor_copy(
                            xT[:, kt, bg * P:(bg + 1) * P], ps[:])
                return xT

            def u_piece(xT, U, mt):
                """One M-tile of the U-chunk matmul + bias copy."""
                psu = up.tile([P, UCOLS], F32, tag="upsum")
                nc.tensor.matmul(psu[:], WXs[:, 0, mt, :], xT[:, 0, :],
                                 start=True, stop=False)
                nc.tensor.matmul(psu[:], WXs[:, 1, mt, :], xT[:, 1, :],
                                 start=False, stop=True)
                nc.vector.tensor_scalar(
                    out=U[:, mt, :, :].rearrange("p b t -> p (b t)"),
                    in0=psu[:], scalar1=BS[:, mt:mt + 1],
                    scalar2=None, op0=ALU.add)

            def step(t, U, cT, cT_prev_buf, g_step):
                """One LSTM timestep (t = index within chunk)."""
                h_prev = hX if g_step % 2 == 0 else hY
                h_new = hY if g_step % 2 == 0 else hX
                c_prev = (cT[:, :, :, t - 1] if t > 0
                          else cT_prev_buf[:, :, :, CH - 1])

                zps = zp.tile([P, 64], F32, tag="zps")
                for mt in range(8):
                    for kt in range(2):
                        nc.tensor.matmul(
                            zps[:, mt * 8:(mt + 1) * 8],
                            WHs[:, kt, mt, :],
                            h_prev[:, kt, :],
                            start=(kt == 0), stop=(kt == 1))

                zbuf = sp.tile([P, 4, 2, B_LOC], F32, tag="zbuf")
                uslice = U[:, :, :, t].rearrange("p (g h) b -> p g h b", h=2)
                nc.vector.tensor_tensor(
                    zbuf[:], zps[:].rearrange("p (g h b) -> p g h b", h=2, b=B_LOC),
                    uslice, ALU.add)

                sg = sp.tile([P, 4, 2, B_LOC], F32, tag="sg")
                nc.scalar.activation(sg[:], zbuf[:], AF.Sigmoid)
                gi, gj, gf, go = sg[:, 0], sg[:, 1], sg[:, 2], sg[:, 3]

                t1 = sp.tile([P, 2, B_LOC], F32, tag="t1")
                nc.vector.tensor_mul(t1[:], gi, gj)
                d = sp.tile([P, 2, B_LOC], F32, tag="d")
                nc.vector.scalar_tensor_tensor(
                    d[:], t1[:], 2.0, gi, ALU.mult, ALU.subtract)
                ca = sp.tile([P, 2, B_LOC], F32, tag="ca")
                nc.gpsimd.tensor_mul(ca[:], c_prev, gf)
                nc.vector.tensor_add(cT[:, :, :, t], ca[:], d[:])

                th = sp.tile([P, 2, B_LOC], F32, tag="th")
                nc.scalar.activation(th[:], cT[:, :, :, t], AF.Tanh)
                nc.vector.tensor_mul(h_new[:], th[:], go)
                return h_new

            def out_store(oT, t0_sv):
                """Transpose the chunk's h-history and DMA to `out`."""
                for bg in range(2):
                    stg = xp.tile([P, H], F32, tag=f"stg{bg}")
                    for bl in range(4):
                        b = bg * 4 + bl
                        for half in range(2):
                            ps = tp.tile([CH, P], F32, tag="otps")
                            nc.tensor.transpose(ps[:], oT[:, half, b, :], IDs[:])
                            nc.vector.tensor_copy(
                                stg[bl * CH:(bl + 1) * CH,
                                    half * P:(half + 1) * P], ps[:])
                    dst = out[bg * 4:(bg + 1) * 4, :, :][:, ds(t0_sv, CH), :]
                    nc.sync.dma_start(dst, stg[:])

            def write_oT(oT, t, h_new):
                nc.gpsimd.tensor_copy(oT[:, :, :, t], h_new[:])

            # ---- prologue: build U for chunk 0 ----
            xt0 = dma_x_chunk(0)
            xT0 = make_xT(xt0)
            for mt in range(8):
                u_piece(xT0, UA, mt)

            # ---- main loop: one iteration = chunks (2i, 2i+1) ----
            with tc.For_i(0, ITERS, 1,
                          hint_engines=(mybir.EngineType.PE,
                                        mybir.EngineType.DVE)) as it:
                t0e = it * (2 * CH)
                t0o = it * (2 * CH) + CH
                t0n = it * (2 * CH) + 2 * CH

                # even chunk; build UB (for the odd chunk) alongside
                xt_o = dma_x_chunk(t0o)
                xT_o = None
                for t in range(CH):
                    hn = step(t, UA, cTe, cTo, t)
                    write_oT(oTe, t, hn)
                    if t == 2:
                        xT_o = make_xT(xt_o)
                    if 4 <= t < 12:
                        u_piece(xT_o, UB, t - 4)

                # odd chunk; build UA for the next iteration's even chunk,
                # and store the even chunk's outputs alongside
                xt_n = dma_x_chunk(t0n)
                xT_n = None
                for t in range(CH):
                    hn = step(t, UB, cTo, cTe, t)
                    write_oT(oTo, t, hn)
                    if t == 2:
                        xT_n = make_xT(xt_n)
                    if 4 <= t < 12:
                        u_piece(xT_n, UA, t - 4)
                    if t == 14:
                        out_store(oTe, t0e)

                out_store(oTo, t0o)
                nc.sync.dma_start(
                    c_raw[ds(it * 2, 1)],
                    cTe[:].rearrange("p h b t -> p (h b t)"))
                nc.sync.dma_start(
                    c_raw[ds(it * 2 + 1, 1)],
                    cTo[:].rearrange("p h b t -> p (h b t)"))

    return nc


def _prep_weights(W, b):
    """Host-side weight prep: split/scale/reorder into device layouts."""
    W = np.asarray(W, np.float32)
    b = np.asarray(b, np.float32)
    Wx = W[:E].copy()            # [256, 1024]
    Wh = W[E:].copy()            # [256, 1024]
    b_eff = b.copy()
    # j gate (cols 256:512) pre-scaled by 2 for tanh(j) = 2*sigmoid(2j) - 1
    Wx[:, 256:512] *= 2.0
    Wh[:, 256:512] *= 2.0
    b_eff[256:512] *= 2.0
    # f gate (cols 512:768): reference uses sigmoid(f + 1)
    b_eff[512:768] += 1.0

    # Build [2, 8, 128, 128] tile arrays: arr[kt, mt] = M[kt*128:(kt+1)*128, mt*128:(mt+1)*128]
    def to_tiles(M, dt):
        a = M.reshape(2, P, 8, P).transpose(0, 2, 1, 3)
        return np.ascontiguousarray(a).astype(dt)

    wx_t = to_tiles(Wx, np.float32)
    wh_t = to_tiles(Wh, np.float16)
    bcol = np.ascontiguousarray(b_eff.reshape(8, P)).astype(np.float32)
    return wx_t, wh_t, bcol


def kernel(input_embeddings, input_length, W, b):
    from concourse.bass_utils import run_bass_kernel_spmd

    x = np.asarray(input_embeddings, np.float32)
    lengths = np.asarray(input_length, np.int32)
    B, T, _ = x.shape
    assert B == N_CORES * B_LOC

    key = T
    if key not in _CACHE:
        nc_new = build_nc(T)
        if not nc_new.is_finalized():
            nc_new.finalize()
        _CACHE[key] = nc_new
    nc = _CACHE[key]

    wx_t, wh_t, bcol = _prep_weights(W, b)
    ident = np.eye(P, dtype=np.float32)
    TPAD = T + 2 * CH

    in_maps = []
    for k in range(N_CORES):
        xs = np.zeros((B_LOC, TPAD, E), np.float32)
        xs[:, :T] = x[k * B_LOC:(k + 1) * B_LOC]
        in_maps.append({
            "x_pad": xs, "wx": wx_t, "wh": wh_t, "bcol": bcol, "ident": ident,
        })

    trace = os.environ.get("KERNEL_PROFILE", "0") == "1"
    kw = {}
    if trace:
        try:
            import profshim
            profshim.install()
            kw["tmpdir"] = os.environ.get("KERNEL_TRACE_DIR") or None
        except Exception:
            trace = False
    try:
        res = run_bass_kernel_spmd(nc, in_maps, list(range(N_CORES)),
                                   trace=trace, **kw)
    except ModuleNotFoundError:
        res = run_bass_kernel_spmd(nc, in_maps, list(range(N_CORES)), trace=False)
    global LAST_EXEC_NS
    LAST_EXEC_NS = res.exec_time_ns

    NCH = T // CH
    outputs = np.concatenate([res.results[k]["out"] for k in range(N_CORES)], axis=0)
    h_fin = np.zeros((B, H), np.float32)
    c_fin = np.zeros((B, H), np.float32)
    for bi in range(B):
        L = int(lengths[bi])
        if L > 0:
            h_fin[bi] = outputs[bi, L - 1]
            core, bl = bi // B_LOC, bi % B_LOC
            craw = res.results[core]["c_raw"]  # [NCH, 128, 512]
            ci, tr = (L - 1) // CH, (L - 1) % CH
            for half in range(2):
                c_fin[bi, half * P:(half + 1) * P] = \
                    craw[ci, :, half * (B_LOC * CH) + bl * CH + tr]
        outputs[bi, L:] = 0.0

    return outputs, (h_fin, c_fin)


LAST_EXEC_NS = None
